# revision 15
# baseline (speedup 1.0000x reference)
"""Trainium2 Bass kernel for nn_DirectDistanceModel.

Host side (index/layout work, as in the original baseline): last-write
winner selection per scatter cell (the reference's scatter semantics),
item_to_loc resolution, compaction of the loc matrix to the rows/cols it
actually contributes through (rows = item locations + start row, cols =
item locations + end col), per-core loc-row slices for the core's 250
items, the transposed seq-winner matrix, and the one-hot column-selection
matrix B[j, c] = [c_j == c] (exact in fp8 e4m3).

Device side (8 NeuronCores, SPMD): the join
    comp1 = sum_{i,j} seq[i,j] * loc[r_i, c_j]
is computed without any gather as
    A = seq^T-tiles x B   (tensor engine, fp8 in / f32 PSUM accumulate)
    comp1 = sum A .* lrows (vector engine)
since A[i, c] = sum_j seq[i,j]*[c_j == c]. The start-depot row rides as a
virtual item whose seq column is all-ones (comp2), and the end-depot
column is a slice of lrows (comp3). Partials are reduced across
partitions with a ones-matmul, AllReduced across the 8 cores, and the
replicated 3->32->1 MLP produces the output.
"""
import numpy as np

N_ITEMS = 2000
N_STORAGE = 4094
N_LOCS = 4096
N_CORES = 8

IPC = 250            # items per core
NT = 16              # seq j-tiles (16 x 128 = 2048 >= N_ITEMS)

_CACHE = {}


def _last_write_winners(cells, order_vals):
    """Last occurrence per unique cell value (stable sort by cell)."""
    order = np.argsort(cells, kind="stable")
    c_sorted = cells[order]
    n = len(order)
    if n == 0:
        return np.empty(0, cells.dtype), np.empty(0, np.float32)
    last = np.empty(n, bool)
    last[:-1] = c_sorted[1:] != c_sorted[:-1]
    last[-1] = True
    return c_sorted[last], order_vals[order][last]


def _host_prep(edge_index, edge_attr, edge_type_mask):
    import ml_dtypes

    src = np.asarray(edge_index[0], dtype=np.int64)
    dst = np.asarray(edge_index[1], dtype=np.int64)
    mask = np.asarray(edge_type_mask, dtype=bool)
    attr = np.asarray(edge_attr, dtype=np.float32)

    # ---- item -> storage loc (type 2) ----
    li = dst - N_ITEMS
    v2 = mask[:, 2] & (src >= 0) & (src < N_ITEMS) & (li >= 0) & (li < N_STORAGE)
    i2 = np.flatnonzero(v2)
    w2_item, w2_loc = _last_write_winners(src[i2], li[i2].astype(np.float32))
    itl = np.zeros(N_ITEMS, np.int64)
    itl[w2_item] = w2_loc.astype(np.int64)

    # ---- compaction tables ----
    R = np.unique(np.append(itl, N_STORAGE))
    C = np.unique(np.append(itl, N_STORAGE + 1))
    nR, nC = len(R), len(C)
    rankR = np.full(N_LOCS, -1, np.int64)
    rankR[R] = np.arange(nR)
    rankC = np.full(N_LOCS, -1, np.int64)
    rankC[C] = np.arange(nC)

    # ---- loc-loc winners (type 0), filtered to referenced rows/cols ----
    ls = src - N_ITEMS
    ld = dst - N_ITEMS
    v0 = mask[:, 0] & (ls >= 0) & (ls < N_LOCS) & (ld >= 0) & (ld < N_LOCS)
    i0 = np.flatnonzero(v0)
    rr = rankR[ls[i0]]
    cc = rankC[ld[i0]]
    keep = (rr >= 0) & (cc >= 0)
    w0_cell, w0_val = _last_write_winners(rr[keep] * nC + cc[keep],
                                          attr[i0[keep], 0])

    # ---- item-item winners (type 1) ----
    v1 = mask[:, 1] & (src >= 0) & (src < N_ITEMS) & (dst >= 0) & (dst < N_ITEMS)
    i1 = np.flatnonzero(v1)
    w1_cell, w1_val = _last_write_winners(src[i1] * N_ITEMS + dst[i1],
                                          attr[i1, 1])

    # ---- dense compact matrices (index-resolved winner placement) ----
    NCpad = -(-(nC + 2) // 4) * 4        # 4-aligned, >= nC+2 zero pad cols
    Lc = np.zeros((nR, NCpad), np.float32)
    Lc[w0_cell // nC, w0_cell % nC] = w0_val
    Sq = np.zeros((N_ITEMS, N_ITEMS), np.float32)
    Sq[w1_cell // N_ITEMS, w1_cell % N_ITEMS] = w1_val

    ri = rankR[itl]                      # compact row of each item
    cj = rankC[itl]                      # compact col of each item
    start_row = rankR[N_STORAGE]

    # ---- one-hot B tiles: bt[p, t*NCpad + c] = [ c_{t*128+p} == c ] ----
    bt = np.zeros((128, NT * NCpad), ml_dtypes.float8_e4m3)
    j = np.arange(N_ITEMS)
    bt[j % 128, (j // 128) * NCpad + cj] = 1.0

    in_maps = []
    nrem = IPC - 128                     # items in chunk 1 (cols 129..)
    for c in range(N_CORES):
        m = {}
        items = np.arange(c * IPC, (c + 1) * IPC)
        # seq^T tiles: M[j, k] = seq[item_of_col_k, j]; col 128 = virtual
        # all-ones column (start depot)
        M = np.zeros((NT * 128, 256), np.float32)
        M[:N_ITEMS, 0:128] = Sq[items[:128], :].T
        M[:N_ITEMS, 128] = 1.0
        M[:N_ITEMS, 129:129 + nrem] = Sq[items[128:], :].T
        st = M.reshape(NT, 128, 256).transpose(1, 0, 2).reshape(128, NT * 256)
        m["seqt"] = st.astype(ml_dtypes.float8_e4m3)

        lrows0 = np.ascontiguousarray(Lc[ri[items[:128]]])
        lrows1 = np.zeros((128, NCpad), np.float32)
        lrows1[0] = Lc[start_row]        # virtual start-depot row
        lrows1[1:1 + nrem] = Lc[ri[items[128:]]]
        m3m0 = np.ones((128, 1), np.float32)
        m3m1 = np.zeros((128, 1), np.float32)
        m3m1[1:1 + nrem, 0] = 1.0
        m["lrows0"], m["lrows1"] = lrows0, lrows1
        m["m3m0"], m["m3m1"] = m3m0, m3m1
        m["btiles"] = bt
        in_maps.append(m)
    return in_maps, NCpad, nC


def _build(NCpad, nC):
    import concourse.bass as bass
    import concourse.mybir as mybir
    from concourse.tile import TileContext

    F32 = mybir.dt.float32
    F8 = mybir.dt.float8e4
    CW = NCpad // 4

    nc = bass.Bass("TRN2")
    p = {}
    p["btiles"] = nc.declare_dram_parameter("btiles", [128, NT * NCpad], F8, isOutput=False)
    p["seqt"] = nc.declare_dram_parameter("seqt", [128, NT * 256], F8, isOutput=False)
    for b in range(2):
        p[f"lrows{b}"] = nc.declare_dram_parameter(f"lrows{b}", [128, NCpad], F32, isOutput=False)
        p[f"m3m{b}"] = nc.declare_dram_parameter(f"m3m{b}", [128, 1], F32, isOutput=False)
    p["W1"] = nc.declare_dram_parameter("W1", [3, 32], F32, isOutput=False)
    p["b1"] = nc.declare_dram_parameter("b1", [1, 32], F32, isOutput=False)
    p["W2"] = nc.declare_dram_parameter("W2", [32, 1], F32, isOutput=False)
    p["b2"] = nc.declare_dram_parameter("b2", [1, 1], F32, isOutput=False)
    pred = nc.declare_dram_parameter("pred", [1, 1], F32, isOutput=True)

    ar_in = nc.dram_tensor("ar_in", [1, 8], F32)
    ar_out = nc.dram_tensor("ar_out", [1, 8], F32, addr_space="Shared")

    with TileContext(nc) as tc:
        with (
            tc.tile_pool(name="p", bufs=1) as pool,
            tc.tile_pool(name="ps", bufs=1, space="PSUM") as psp,
        ):
            # ---------- loads (fine-grained, spread across HWDGE queues) --
            bt = pool.tile([128, NT * NCpad], F8, tag="bt")
            st = pool.tile([128, NT * 256], F8, tag="st")
            nc.sync.dma_start(out=st[:, 0:4 * 256], in_=p["seqt"][:, 0:4 * 256])
            nc.scalar.dma_start(out=bt[:, 0:NCpad], in_=p["btiles"][:, 0:NCpad])
            for t in range(1, NT):
                eng = nc.scalar if t % 2 == 0 else nc.sync
                eng.dma_start(
                    out=bt[:, t * NCpad:(t + 1) * NCpad],
                    in_=p["btiles"][:, t * NCpad:(t + 1) * NCpad])
            for q in range(1, 4):
                nc.sync.dma_start(out=st[:, q * 4 * 256:(q + 1) * 4 * 256],
                                  in_=p["seqt"][:, q * 4 * 256:(q + 1) * 4 * 256])
            lr = []
            m3 = []
            for b in range(2):
                t_ = pool.tile([128, NCpad], F32, tag=f"lr{b}")
                nc.scalar.dma_start(out=t_[:, :], in_=p[f"lrows{b}"][:, :])
                lr.append(t_)
                t_ = pool.tile([128, 1], F32, tag=f"m3_{b}")
                nc.sync.dma_start(out=t_[:, :], in_=p[f"m3m{b}"][:, :])
                m3.append(t_)

            r1 = pool.tile([128, 1], F32, tag="r1")
            parts = pool.tile([128, 3], F32, tag="parts")
            nc.vector.memset(r1[:, :], 0.0)
            nc.vector.memset(parts[:, :], 0.0)

            psA0 = psp.tile([128, CW], F32, tag="psA0")
            psA1 = psp.tile([128, CW], F32, tag="psA1")
            psA2 = psp.tile([128, CW], F32, tag="psA2")
            psA3 = psp.tile([128, CW], F32, tag="psA3")
            psA = [psA0, psA1, psA2, psA3]
            prod = pool.tile([128, NCpad], F32, tag="prod")
            for chunk in range(2):
                for t in range(NT):
                    lhs = st[:, t * 256 + chunk * 128:t * 256 + chunk * 128 + 128]
                    for q in range(4):
                        nc.tensor.matmul(
                            psA[q][:, :], lhs,
                            bt[:, t * NCpad + q * CW:t * NCpad + (q + 1) * CW],
                            start=(t == 0), stop=(t == NT - 1))
                for q in range(4):
                    nc.vector.tensor_mul(
                        out=prod[:, q * CW:(q + 1) * CW], in0=psA[q][:, :],
                        in1=lr[chunk][:, q * CW:(q + 1) * CW])
                rb = pool.tile([128, 1], F32, tag=f"rb{chunk}")
                nc.vector.tensor_reduce(rb[:, :], prod[:, :],
                                        mybir.AxisListType.X,
                                        mybir.AluOpType.add)
                if chunk == 1:
                    # partition 0 of chunk 1 is the virtual start row; comp2
                    # is computed in full on every core -> pre-scale by 1/8
                    nc.vector.tensor_scalar(out=parts[0:1, 1:2],
                                            in0=rb[0:1, :], scalar1=0.125,
                                            scalar2=None,
                                            op0=mybir.AluOpType.mult)
                    nc.vector.tensor_mul(out=rb[:, :], in0=rb[:, :],
                                         in1=m3[1][:, :])
                nc.vector.tensor_add(out=r1[:, :], in0=r1[:, :], in1=rb[:, :])
                # comp3: end-depot column times item mask
                endv = pool.tile([128, 1], F32, tag=f"endv{chunk}")
                nc.vector.tensor_mul(out=endv[:, :],
                                     in0=lr[chunk][:, nC - 1:nC],
                                     in1=m3[chunk][:, :])
                nc.vector.tensor_add(out=parts[:, 2:3], in0=parts[:, 2:3],
                                     in1=endv[:, :])

            nc.vector.tensor_copy(out=parts[:, 0:1], in_=r1[:, :])

            # ---------- reduce across partitions + AllReduce ----------
            ones = pool.tile([128, 1], F32, tag="ones")
            nc.vector.memset(ones[:, :], 1.0)
            psum3 = psp.tile([1, 3], F32, tag="psum3")
            nc.tensor.matmul(psum3[:, :], ones[:, :], parts[:, :],
                             start=True, stop=True)
            packed = pool.tile([1, 8], F32, tag="packed")
            nc.vector.memset(packed[:, :], 0.0)
            nc.vector.tensor_copy(out=packed[0:1, 0:3], in_=psum3[0:1, 0:3])
            nc.sync.dma_start(out=ar_in[:, :], in_=packed[:, :])
            nc.gpsimd.collective_compute(
                "AllReduce",
                mybir.AluOpType.add,
                replica_groups=[list(range(N_CORES))],
                ins=[ar_in[:, :]],
                outs=[ar_out[:, :]],
            )

            # ---------- MLP (bias folded into matmul, b2 into the dot) ----
            # comps4 = [c1 c2 c3 1]^T; w1b = [W1; b1] so hpsum = c.W1 + b1.
            comps4 = pool.tile([4, 1], F32, tag="comps4")
            nc.vector.memset(comps4[:, :], 1.0)
            nc.sync.dma_start(out=comps4[0:3, :],
                              in_=ar_out[0:1, 0:3].rearrange("one k -> k one"))
            w1b = pool.tile([4, 32], F32, tag="w1b")
            nc.sync.dma_start(out=w1b[0:3, :], in_=p["W1"][:, :])
            nc.sync.dma_start(out=w1b[3:4, :], in_=p["b1"][:, :])
            hpsum = psp.tile([1, 32], F32, tag="hpsum")
            nc.tensor.matmul(hpsum[:, :], comps4[:, :], w1b[:, :],
                             start=True, stop=True)
            # hr[0:32] = relu(h); hr[32] = 1 so the dot with [W2; b2] adds b2.
            hr = pool.tile([1, 33], F32, tag="hr")
            nc.vector.memset(hr[:, :], 1.0)
            w2b = pool.tile([1, 33], F32, tag="w2b")
            nc.sync.dma_start(out=w2b[0:1, 0:32],
                              in_=p["W2"][:, :].rearrange("k one -> one k"))
            nc.sync.dma_start(out=w2b[0:1, 32:33], in_=p["b2"][:, :])
            nc.vector.tensor_relu(out=hr[0:1, 0:32], in_=hpsum[:, :])
            hw = pool.tile([1, 33], F32, tag="hw")
            nc.vector.tensor_mul(out=hw[:, :], in0=hr[:, :], in1=w2b[:, :])
            out1 = pool.tile([1, 1], F32, tag="out1")
            nc.vector.tensor_reduce(out1[:, :], hw[:, :], mybir.AxisListType.X,
                                    mybir.AluOpType.add)
            nc.sync.dma_start(out=pred[:, :], in_=out1[:, :])

    _split_sync_waits(nc)
    return nc


def _split_sync_waits(nc, max_waits=1):
    import concourse.mybir as mybir
    ctr = [0]
    for f in nc.m.functions:
        for bb in f.blocks:
            new_insts = []
            for inst in bb.instructions:
                si = getattr(inst, "sync_info", None)
                if si is not None and si.on_wait and len(si.on_wait) > max_waits:
                    waits = list(si.on_wait)
                    head, tail = waits[:-max_waits], waits[-max_waits:]
                    while head:
                        chunk, head = head[:max_waits], head[max_waits:]
                        ctr[0] += 1
                        nop = mybir.InstNoOp(
                            name=f"I-syncfix-{ctr[0]}",
                            engine=inst.engine,
                            ins=[],
                            outs=[],
                            sync_info=mybir.SyncInfo(on_wait=chunk,
                                                     on_update=[]),
                            bass_nofuse=True,
                        )
                        new_insts.append(nop)
                    inst.sync_info = mybir.SyncInfo(
                        on_wait=tail, on_update=list(si.on_update))
                new_insts.append(inst)
            bb.instructions[:] = new_insts


def kernel(**inputs):
    import os
    from concourse.bass_utils import run_bass_kernel_spmd

    edge_index = np.asarray(inputs["edge_index"])
    edge_attr = np.asarray(inputs["edge_attr"])
    edge_type_mask = np.asarray(inputs["edge_type_mask"])
    assert int(inputs["n_items"]) == N_ITEMS
    assert int(inputs["n_storage"]) == N_STORAGE
    assert int(inputs["n_locs"]) == N_LOCS

    in_maps, NCpad, nC = _host_prep(edge_index, edge_attr, edge_type_mask)
    W1 = np.asarray(inputs["W1"], np.float32).reshape(3, 32)
    b1 = np.asarray(inputs["b1"], np.float32).reshape(1, 32)
    W2 = np.asarray(inputs["W2"], np.float32).reshape(32, 1)
    b2 = np.asarray(inputs["b2"], np.float32).reshape(1, 1)
    for m in in_maps:
        m["W1"] = W1
        m["b1"] = b1
        m["W2"] = W2
        m["b2"] = b2

    key = (NCpad, nC)
    if key not in _CACHE:
        _CACHE[key] = _build(NCpad, nC)
    nc = _CACHE[key]
    trace = os.environ.get("KERNEL_TRACE") == "1"
    res = run_bass_kernel_spmd(nc, in_maps, core_ids=list(range(N_CORES)),
                               trace=trace)
    if trace and res.exec_time_ns is not None:
        print(f"HW exec time: {res.exec_time_ns} ns")
    out = res.results[0]["pred"]
    return np.float32(out.reshape(())).astype(np.float32)


# revision 16
# speedup vs baseline: 9.1260x; 9.1260x over previous
"""Trainium2 Bass kernel for nn_DirectDistanceModel.

Host side (index/layout work, as in the original baseline): last-write
winner selection per scatter cell (the reference's scatter semantics),
item_to_loc resolution, compaction of the loc matrix to the rows/cols it
actually contributes through (rows = item locations + start row, cols =
item locations + end col), per-core loc-row slices for the core's 250
items, the transposed seq-winner matrix, and the one-hot column-selection
matrix B[j, c] = [c_j == c] (exact in fp8 e4m3).

Device side (8 NeuronCores, SPMD): the join
    comp1 = sum_{i,j} seq[i,j] * loc[r_i, c_j]
is computed without any gather as
    A = seq^T-tiles x B   (tensor engine, fp8 in / f32 PSUM accumulate)
    comp1 = sum A .* lrows (vector engine)
since A[i, c] = sum_j seq[i,j]*[c_j == c]. The start-depot row rides as a
virtual item whose seq column is all-ones (comp2), and the end-depot
column is a slice of lrows (comp3). Partials are reduced across
partitions with a ones-matmul, AllReduced across the 8 cores, and the
replicated 3->32->1 MLP produces the output.
"""
import numpy as np

N_ITEMS = 2000
N_STORAGE = 4094
N_LOCS = 4096
N_CORES = 8

IPC = 250            # items per core
NT = 16              # seq j-tiles (16 x 128 = 2048 >= N_ITEMS)

_CACHE = {}


def _last_write_winners(cells, order_vals):
    """Last occurrence per unique cell value (stable sort by cell)."""
    order = np.argsort(cells, kind="stable")
    c_sorted = cells[order]
    n = len(order)
    if n == 0:
        return np.empty(0, cells.dtype), np.empty(0, np.float32)
    last = np.empty(n, bool)
    last[:-1] = c_sorted[1:] != c_sorted[:-1]
    last[-1] = True
    return c_sorted[last], order_vals[order][last]


def _host_prep(edge_index, edge_attr, edge_type_mask):
    import ml_dtypes

    src = np.asarray(edge_index[0], dtype=np.int64)
    dst = np.asarray(edge_index[1], dtype=np.int64)
    mask = np.asarray(edge_type_mask, dtype=bool)
    attr = np.asarray(edge_attr, dtype=np.float32)

    # ---- item -> storage loc (type 2) ----
    li = dst - N_ITEMS
    v2 = mask[:, 2] & (src >= 0) & (src < N_ITEMS) & (li >= 0) & (li < N_STORAGE)
    i2 = np.flatnonzero(v2)
    w2_item, w2_loc = _last_write_winners(src[i2], li[i2].astype(np.float32))
    itl = np.zeros(N_ITEMS, np.int64)
    itl[w2_item] = w2_loc.astype(np.int64)

    # ---- compaction tables ----
    R = np.unique(np.append(itl, N_STORAGE))
    C = np.unique(np.append(itl, N_STORAGE + 1))
    nR, nC = len(R), len(C)
    rankR = np.full(N_LOCS, -1, np.int64)
    rankR[R] = np.arange(nR)
    rankC = np.full(N_LOCS, -1, np.int64)
    rankC[C] = np.arange(nC)

    # ---- loc-loc winners (type 0), filtered to referenced rows/cols ----
    ls = src - N_ITEMS
    ld = dst - N_ITEMS
    v0 = mask[:, 0] & (ls >= 0) & (ls < N_LOCS) & (ld >= 0) & (ld < N_LOCS)
    i0 = np.flatnonzero(v0)
    rr = rankR[ls[i0]]
    cc = rankC[ld[i0]]
    keep = (rr >= 0) & (cc >= 0)
    w0_cell, w0_val = _last_write_winners(rr[keep] * nC + cc[keep],
                                          attr[i0[keep], 0])

    # ---- item-item winners (type 1) ----
    v1 = mask[:, 1] & (src >= 0) & (src < N_ITEMS) & (dst >= 0) & (dst < N_ITEMS)
    i1 = np.flatnonzero(v1)
    w1_cell, w1_val = _last_write_winners(src[i1] * N_ITEMS + dst[i1],
                                          attr[i1, 1])

    # ---- dense compact matrices (index-resolved winner placement) ----
    NCpad = -(-(nC + 2) // 4) * 4        # 4-aligned, >= nC+2 zero pad cols
    Lc = np.zeros((nR, NCpad), np.float32)
    Lc[w0_cell // nC, w0_cell % nC] = w0_val
    Sq = np.zeros((N_ITEMS, N_ITEMS), np.float32)
    Sq[w1_cell // N_ITEMS, w1_cell % N_ITEMS] = w1_val

    ri = rankR[itl]                      # compact row of each item
    cj = rankC[itl]                      # compact col of each item
    start_row = rankR[N_STORAGE]

    # ---- one-hot B tiles: bt[p, t*NCpad + c] = [ c_{t*128+p} == c ] ----
    bt = np.zeros((128, NT * NCpad), ml_dtypes.float8_e4m3)
    j = np.arange(N_ITEMS)
    bt[j % 128, (j // 128) * NCpad + cj] = 1.0

    in_maps = []
    nrem = IPC - 128                     # items in chunk 1 (cols 129..)
    for c in range(N_CORES):
        m = {}
        items = np.arange(c * IPC, (c + 1) * IPC)
        # seq^T tiles: M[j, k] = seq[item_of_col_k, j]; col 128 = virtual
        # all-ones column (start depot)
        M = np.zeros((NT * 128, 256), np.float32)
        M[:N_ITEMS, 0:128] = Sq[items[:128], :].T
        M[:N_ITEMS, 128] = 1.0
        M[:N_ITEMS, 129:129 + nrem] = Sq[items[128:], :].T
        st = M.reshape(NT, 128, 256).transpose(1, 0, 2).reshape(128, NT * 256)
        m["seqt"] = st.astype(ml_dtypes.float8_e4m3)

        lrows0 = np.ascontiguousarray(Lc[ri[items[:128]]])
        lrows1 = np.zeros((128, NCpad), np.float32)
        lrows1[0] = Lc[start_row]        # virtual start-depot row
        lrows1[1:1 + nrem] = Lc[ri[items[128:]]]
        m3m0 = np.ones((128, 1), np.float32)
        m3m1 = np.zeros((128, 1), np.float32)
        m3m1[1:1 + nrem, 0] = 1.0
        m["lrows0"], m["lrows1"] = lrows0, lrows1
        m["m3m0"], m["m3m1"] = m3m0, m3m1
        m["btiles"] = bt
        in_maps.append(m)
    return in_maps, NCpad, nC


def _build(NCpad, nC):
    import concourse.bass as bass
    import concourse.mybir as mybir
    from concourse.tile import TileContext

    F32 = mybir.dt.float32
    F8 = mybir.dt.float8e4
    CW = NCpad // 4

    nc = bass.Bass("TRN2")
    p = {}
    p["btiles"] = nc.declare_dram_parameter("btiles", [128, NT * NCpad], F8, isOutput=False)
    p["seqt"] = nc.declare_dram_parameter("seqt", [128, NT * 256], F8, isOutput=False)
    for b in range(2):
        p[f"lrows{b}"] = nc.declare_dram_parameter(f"lrows{b}", [128, NCpad], F32, isOutput=False)
        p[f"m3m{b}"] = nc.declare_dram_parameter(f"m3m{b}", [128, 1], F32, isOutput=False)
    p["W1"] = nc.declare_dram_parameter("W1", [3, 32], F32, isOutput=False)
    p["b1"] = nc.declare_dram_parameter("b1", [1, 32], F32, isOutput=False)
    p["W2"] = nc.declare_dram_parameter("W2", [32, 1], F32, isOutput=False)
    p["b2"] = nc.declare_dram_parameter("b2", [1, 1], F32, isOutput=False)
    pred = nc.declare_dram_parameter("pred", [1, 1], F32, isOutput=True)

    ar_in = nc.dram_tensor("ar_in", [1, 8], F32)
    ar_out = nc.dram_tensor("ar_out", [1, 8], F32, addr_space="Shared")

    with TileContext(nc) as tc:
        with (
            tc.tile_pool(name="p", bufs=1) as pool,
            tc.tile_pool(name="ps", bufs=1, space="PSUM") as psp,
        ):
            # ---------- loads (spread across HWDGE queues) ----------
            bt = pool.tile([128, NT * NCpad], F8, tag="bt")
            st = pool.tile([128, NT * 256], F8, tag="st")
            nc.sync.dma_start(out=st[:, :], in_=p["seqt"][:, :])
            for q in range(4):
                eng = nc.scalar if q < 2 else nc.sync
                eng.dma_start(
                    out=bt[:, q * 4 * NCpad:(q + 1) * 4 * NCpad],
                    in_=p["btiles"][:, q * 4 * NCpad:(q + 1) * 4 * NCpad])
            lr = []
            m3 = []
            for b in range(2):
                t_ = pool.tile([128, NCpad], F32, tag=f"lr{b}")
                nc.scalar.dma_start(out=t_[:, :], in_=p[f"lrows{b}"][:, :])
                lr.append(t_)
                t_ = pool.tile([128, 1], F32, tag=f"m3_{b}")
                nc.sync.dma_start(out=t_[:, :], in_=p[f"m3m{b}"][:, :])
                m3.append(t_)

            r1 = pool.tile([128, 1], F32, tag="r1")
            parts = pool.tile([128, 3], F32, tag="parts")
            nc.vector.memset(r1[:, :], 0.0)
            nc.vector.memset(parts[:, :], 0.0)

            psA0 = psp.tile([128, CW], F32, tag="psA0")
            psA1 = psp.tile([128, CW], F32, tag="psA1")
            psA2 = psp.tile([128, CW], F32, tag="psA2")
            psA3 = psp.tile([128, CW], F32, tag="psA3")
            psA = [psA0, psA1, psA2, psA3]
            prod = pool.tile([128, NCpad], F32, tag="prod")
            for chunk in range(2):
                for t in range(NT):
                    lhs = st[:, t * 256 + chunk * 128:t * 256 + chunk * 128 + 128]
                    for q in range(4):
                        nc.tensor.matmul(
                            psA[q][:, :], lhs,
                            bt[:, t * NCpad + q * CW:t * NCpad + (q + 1) * CW],
                            start=(t == 0), stop=(t == NT - 1))
                for q in range(4):
                    nc.vector.tensor_mul(
                        out=prod[:, q * CW:(q + 1) * CW], in0=psA[q][:, :],
                        in1=lr[chunk][:, q * CW:(q + 1) * CW])
                rb = pool.tile([128, 1], F32, tag=f"rb{chunk}")
                nc.vector.tensor_reduce(rb[:, :], prod[:, :],
                                        mybir.AxisListType.X,
                                        mybir.AluOpType.add)
                if chunk == 1:
                    # partition 0 of chunk 1 is the virtual start row
                    nc.vector.tensor_copy(out=parts[0:1, 1:2], in_=rb[0:1, :])
                    nc.vector.tensor_mul(out=rb[:, :], in0=rb[:, :],
                                         in1=m3[1][:, :])
                nc.vector.tensor_add(out=r1[:, :], in0=r1[:, :], in1=rb[:, :])
                # comp3: end-depot column times item mask
                endv = pool.tile([128, 1], F32, tag=f"endv{chunk}")
                nc.vector.tensor_mul(out=endv[:, :],
                                     in0=lr[chunk][:, nC - 1:nC],
                                     in1=m3[chunk][:, :])
                nc.vector.tensor_add(out=parts[:, 2:3], in0=parts[:, 2:3],
                                     in1=endv[:, :])

            nc.vector.tensor_copy(out=parts[:, 0:1], in_=r1[:, :])

            # ---------- reduce across partitions + AllReduce ----------
            ones = pool.tile([128, 1], F32, tag="ones")
            nc.vector.memset(ones[:, :], 1.0)
            psum3 = psp.tile([1, 3], F32, tag="psum3")
            nc.tensor.matmul(psum3[:, :], ones[:, :], parts[:, :],
                             start=True, stop=True)
            packed = pool.tile([1, 8], F32, tag="packed")
            nc.vector.memset(packed[:, :], 0.0)
            nc.vector.tensor_copy(out=packed[0:1, 0:1], in_=psum3[0:1, 0:1])
            # comp2 is computed in full on every core -> scale by 1/8
            nc.vector.tensor_scalar(out=packed[0:1, 1:2],
                                    in0=psum3[0:1, 1:2], scalar1=0.125,
                                    scalar2=None, op0=mybir.AluOpType.mult)
            nc.vector.tensor_copy(out=packed[0:1, 2:3], in_=psum3[0:1, 2:3])
            nc.sync.dma_start(out=ar_in[:, :], in_=packed[:, :])
            nc.gpsimd.collective_compute(
                "AllReduce",
                mybir.AluOpType.add,
                replica_groups=[list(range(N_CORES))],
                ins=[ar_in[:, :]],
                outs=[ar_out[:, :]],
            )

            # ---------- MLP (bias folded into matmul, b2 into the dot) ----
            # comps4 = [c1 c2 c3 1]^T; w1b = [W1; b1] so hpsum = c.W1 + b1.
            comps4 = pool.tile([4, 1], F32, tag="comps4")
            nc.vector.memset(comps4[:, :], 1.0)
            nc.sync.dma_start(out=comps4[0:3, :],
                              in_=ar_out[0:1, 0:3].rearrange("one k -> k one"))
            w1b = pool.tile([4, 32], F32, tag="w1b")
            nc.sync.dma_start(out=w1b[0:3, :], in_=p["W1"][:, :])
            nc.sync.dma_start(out=w1b[3:4, :], in_=p["b1"][:, :])
            hpsum = psp.tile([1, 32], F32, tag="hpsum")
            nc.tensor.matmul(hpsum[:, :], comps4[:, :], w1b[:, :],
                             start=True, stop=True)
            # hr[0:32] = relu(h); hr[32] = 1 so the dot with [W2; b2] adds b2.
            hr = pool.tile([1, 33], F32, tag="hr")
            nc.vector.memset(hr[:, :], 1.0)
            w2b = pool.tile([1, 33], F32, tag="w2b")
            nc.sync.dma_start(out=w2b[0:1, 0:32],
                              in_=p["W2"][:, :].rearrange("k one -> one k"))
            nc.sync.dma_start(out=w2b[0:1, 32:33], in_=p["b2"][:, :])
            nc.vector.tensor_relu(out=hr[0:1, 0:32], in_=hpsum[:, :])
            hw = pool.tile([1, 33], F32, tag="hw")
            nc.vector.tensor_mul(out=hw[:, :], in0=hr[:, :], in1=w2b[:, :])
            out1 = pool.tile([1, 1], F32, tag="out1")
            nc.vector.tensor_reduce(out1[:, :], hw[:, :], mybir.AxisListType.X,
                                    mybir.AluOpType.add)
            nc.sync.dma_start(out=pred[:, :], in_=out1[:, :])

    _split_sync_waits(nc)
    return nc


def _split_sync_waits(nc, max_waits=1):
    import concourse.mybir as mybir
    ctr = [0]
    for f in nc.m.functions:
        for bb in f.blocks:
            new_insts = []
            for inst in bb.instructions:
                si = getattr(inst, "sync_info", None)
                if si is not None and si.on_wait and len(si.on_wait) > max_waits:
                    waits = list(si.on_wait)
                    head, tail = waits[:-max_waits], waits[-max_waits:]
                    while head:
                        chunk, head = head[:max_waits], head[max_waits:]
                        ctr[0] += 1
                        nop = mybir.InstNoOp(
                            name=f"I-syncfix-{ctr[0]}",
                            engine=inst.engine,
                            ins=[],
                            outs=[],
                            sync_info=mybir.SyncInfo(on_wait=chunk,
                                                     on_update=[]),
                            bass_nofuse=True,
                        )
                        new_insts.append(nop)
                    inst.sync_info = mybir.SyncInfo(
                        on_wait=tail, on_update=list(si.on_update))
                new_insts.append(inst)
            bb.instructions[:] = new_insts


def kernel(**inputs):
    import os
    from concourse.bass_utils import run_bass_kernel_spmd

    edge_index = np.asarray(inputs["edge_index"])
    edge_attr = np.asarray(inputs["edge_attr"])
    edge_type_mask = np.asarray(inputs["edge_type_mask"])
    assert int(inputs["n_items"]) == N_ITEMS
    assert int(inputs["n_storage"]) == N_STORAGE
    assert int(inputs["n_locs"]) == N_LOCS

    in_maps, NCpad, nC = _host_prep(edge_index, edge_attr, edge_type_mask)
    W1 = np.asarray(inputs["W1"], np.float32).reshape(3, 32)
    b1 = np.asarray(inputs["b1"], np.float32).reshape(1, 32)
    W2 = np.asarray(inputs["W2"], np.float32).reshape(32, 1)
    b2 = np.asarray(inputs["b2"], np.float32).reshape(1, 1)
    for m in in_maps:
        m["W1"] = W1
        m["b1"] = b1
        m["W2"] = W2
        m["b2"] = b2

    key = (NCpad, nC)
    if key not in _CACHE:
        _CACHE[key] = _build(NCpad, nC)
    nc = _CACHE[key]
    trace = os.environ.get("KERNEL_TRACE") == "1"
    res = run_bass_kernel_spmd(nc, in_maps, core_ids=list(range(N_CORES)),
                               trace=trace)
    if trace and res.exec_time_ns is not None:
        print(f"HW exec time: {res.exec_time_ns} ns")
    out = res.results[0]["pred"]
    return np.float32(out.reshape(())).astype(np.float32)
